# revision 1
# baseline (speedup 1.0000x reference)
"""Trainium2 Bass kernel for nn_Attention_48000554500172.

16-head causal attention with RoPE (S=4096, D=2048, H=16, DH=128), sharded
over heads across 8 NeuronCores (2 heads/core, tensor parallel). Each core
computes its heads' QKV projections, RoPE, causal softmax attention and the
partial output projection; the 8 partial [S, D] outputs are summed on host
(the all-reduce of the sharding hint).

Per-core design:
- x is passed transposed (xT [D, S]); activations live as [dh, s] tiles so
  every matmul contracts over the partition dim with moving free dim 512
  (full-rate float32r).
- All matmuls run in float32r (TF32-like, ~1.5e-4 rel err, full rate at
  free>=256). fp32r operands must be produced by a compute op, so DMA'd
  fp32 data is rounded via ACT/DVE copies.
- RoPE: rotate_half is a position-independent signed pair-swap permutation
  P, applied with a small PE matmul (qp = P @ q), then
  q_rot = q*cosT + qp*sinT on DVE.
- Scores are computed TRANSPOSED (simT [keys, queries]) so no per-tile
  prob transposes are needed before the P@V matmul. Softmax denominators
  (sums over keys = partitions) come from an M=1 ones-matmul accumulated
  in PSUM; normalization is a K=1 broadcast-matmul of 1/Z plus one DVE
  multiply fused with the PV PSUM->SBUF copy. exp() needs no
  max-subtraction (logits ~ N(0,1), |logit| < ~7, fp32 exp safe).
- Causality at 128-key-block granularity; 4 diagonal-block binary mask
  variants zero masked probs post-exp (in-place DVE mul).
"""
import math
import numpy as np
from contextlib import ExitStack

import concourse.bass as bass
import concourse.tile as tile
from concourse import bacc, mybir
from concourse.bass_utils import run_bass_kernel_spmd

D, H, DH = 2048, 16, 128
NCORES = 8
HPC = H // NCORES  # 2 heads per core
ROPE_BASE = 10000.0
SCALE = 1.0 / math.sqrt(DH)
F32 = mybir.dt.float32
F32R = mybir.dt.float32r
Exp = mybir.ActivationFunctionType.Exp

_BUILD_CACHE: dict = {}
TRACE = False          # set True (e.g. from test.py) to capture an NTFF trace
LAST_RESULT = None     # BassKernelResults of the most recent run


def _build(S: int):
    """Emit + compile the per-core Bass program for sequence length S."""
    assert S % 512 == 0
    NSL = S // 512   # s-slices (phase 1)
    ND = D // 128    # 16 contraction tiles
    NG = S // 512    # query groups (phase 2)
    NB = S // 128    # key blocks

    nc = bacc.Bacc("TRN2", target_bir_lowering=False, debug=False)

    xT_d = nc.dram_tensor("xT", [D, S], F32, kind="ExternalInput")
    w_d = nc.dram_tensor("wqkvT", [D, 6 * 128], F32, kind="ExternalInput")
    wo_d = nc.dram_tensor("woT", [2 * DH, D], F32, kind="ExternalInput")
    cs_d = nc.dram_tensor("cs", [128, 2 * S], F32, kind="ExternalInput")
    consts_d = nc.dram_tensor("consts", [128, 257], F32, kind="ExternalInput")
    onesrow_d = nc.dram_tensor("onesrow", [1, 128], F32, kind="ExternalInput")
    out_d = nc.dram_tensor("outp", [S, D], F32, kind="ExternalOutput")

    with tile.TileContext(nc) as tc, ExitStack() as ctx:
        # ---- whole-kernel pools ----
        persist = ctx.enter_context(tc.tile_pool(name="persist", bufs=1))
        constp = ctx.enter_context(tc.tile_pool(name="consts", bufs=1))
        work = ctx.enter_context(tc.tile_pool(name="work", bufs=2))

        # ---- constants ----
        cst_f = constp.tile([128, 257], F32, tag="cstf", name="cstf")
        nc.sync.dma_start(cst_f[:], consts_d.ap())
        PT_r = constp.tile([128, 128], F32R, tag="pt", name="ptr")
        ident_r = constp.tile([128, 128], F32R, tag="ident", name="identr")
        onescol_r = constp.tile([128, 1], F32R, tag="onescol", name="onescolr")
        nc.vector.tensor_copy(PT_r[:], cst_f[:, 0:128])
        nc.vector.tensor_copy(ident_r[:], cst_f[:, 128:256])
        nc.vector.tensor_copy(onescol_r[:], cst_f[:, 256:257])
        onesrow_f = constp.tile([1, 128], F32, tag="onesrowf", name="onesrowf")
        nc.sync.dma_start(onesrow_f[:], onesrow_d.ap())
        onesrow_r = constp.tile([1, 128], F32R, tag="onesrow", name="onesrowr")
        nc.vector.tensor_copy(onesrow_r[:], onesrow_f[:])

        # persistent activations (qT/kT per head, v as [s, dh] blocks)
        qT = [persist.tile([128, S], F32R, tag=f"qT{h}", name=f"qT{h}") for h in range(2)]
        kT = [persist.tile([128, S], F32R, tag=f"kT{h}", name=f"kT{h}") for h in range(2)]
        v_sb = persist.tile([128, NB * 256], F32R, tag="v", name="vsb")

        # ---- phase 1: projections + rope + v transpose ----
        with ExitStack() as ph1:
            wp = ph1.enter_context(tc.tile_pool(name="wp", bufs=1))
            p1w = ph1.enter_context(tc.tile_pool(name="p1w", bufs=2))
            pmm = ph1.enter_context(tc.tile_pool(name="pmm", bufs=6, space="PSUM"))
            pmisc = ph1.enter_context(
                tc.tile_pool(name="pmisc", bufs=2, space="PSUM")
            )

            # qkv weights: [128, d*768 + jt*128], jt = (q0,q1,k0,k1,v0,v1)
            w_r = wp.tile([128, ND * 768], F32R, tag="w", name="wr")
            for d in range(ND):
                wst = p1w.tile([128, 768], F32, tag="wst", bufs=2, name="wst")
                nc.sync.dma_start(wst[:], w_d.ap()[d * 128:(d + 1) * 128, :])
                nc.vector.tensor_copy(w_r[:, d * 768:(d + 1) * 768], wst[:])

            for sl in range(NSL):
                ssl = slice(sl * 512, (sl + 1) * 512)
                cos_sl = p1w.tile([128, 512], F32, tag="cos", bufs=2, name="cossl")
                sin_sl = p1w.tile([128, 512], F32, tag="sin", bufs=2, name="sinsl")
                nc.sync.dma_start(cos_sl[:], cs_d.ap()[:, sl * 512:(sl + 1) * 512])
                nc.sync.dma_start(
                    sin_sl[:], cs_d.ap()[:, S + sl * 512:S + (sl + 1) * 512]
                )

                xr = []
                for dd in range(ND // 2):
                    # fetch two 128-row d-chunks in ONE strided DMA and
                    # round them as one wide op (halves DMA/round op count)
                    xs2 = p1w.tile([128, 1024], F32, tag="xs", bufs=2, name="xs2")
                    src = xT_d.ap()[dd * 256:(dd + 1) * 256, ssl]
                    nc.sync.dma_start(
                        xs2[:].rearrange("b (a c) -> b a c", a=2),
                        src.rearrange("(a b) c -> b a c", a=2),
                    )
                    xrt2 = p1w.tile([128, 1024], F32R, tag="xr", bufs=5, name="xr2")
                    # cycle the fp32r rounding copy across ACT/DVE/GPSIMD
                    if dd % 3 == 0:
                        nc.scalar.copy(xrt2[:], xs2[:])
                    elif dd % 3 == 1:
                        nc.vector.tensor_copy(xrt2[:], xs2[:])
                    else:
                        nc.gpsimd.tensor_copy(xrt2[:], xs2[:])
                    xr.append(xrt2[:, 0:512])
                    xr.append(xrt2[:, 512:1024])

                acc = [
                    pmm.tile([128, 512], F32, tag="mm", bufs=6, name=f"acc{jt}")
                    for jt in range(6)
                ]
                for d in range(ND):
                    for jt in range(6):
                        nc.tensor.matmul(
                            acc[jt][:],
                            w_r[:, d * 768 + jt * 128:d * 768 + (jt + 1) * 128],
                            xr[d],
                            start=(d == 0),
                            stop=(d == ND - 1),
                        )

                for hh in range(2):
                    # rope for q (jt=hh) and k (jt=2+hh); spread the PSUM
                    # drain copies across ACT and DVE so the accumulators
                    # free up quickly for the next slice
                    for jt, dst in ((hh, qT[hh]), (2 + hh, kT[hh])):
                        t_in = p1w.tile([128, 512], F32R, tag="ropein", bufs=2, name="tin")
                        nc.scalar.copy(t_in[:], acc[jt][:])
                        p_ps = pmisc.tile([128, 512], F32, tag="misc", bufs=2, name="pps")
                        nc.tensor.matmul(
                            p_ps[:], PT_r[:], t_in[:], start=True, stop=True
                        )
                        t1 = p1w.tile([128, 512], F32, tag="t1", bufs=2, name="t1")
                        nc.vector.tensor_mul(t1[:], t_in[:], cos_sl[:])
                        t2 = p1w.tile([128, 512], F32, tag="t2", bufs=2, name="t2")
                        nc.vector.tensor_mul(t2[:], p_ps[:], sin_sl[:])
                        nc.vector.tensor_add(dst[:, ssl], t1[:], t2[:])
                    # v: transpose [dh, s] -> [s, dh] 128-blocks
                    vtmp = p1w.tile([128, 512], F32R, tag="vtmp", bufs=2, name="vtmp")
                    nc.scalar.copy(vtmp[:], acc[4 + hh][:])
                    for t in range(4):
                        blk = sl * 4 + t
                        tp = pmisc.tile([128, 128], F32R, tag="misc", bufs=2, name="vtp")
                        nc.tensor.transpose(
                            tp[:], vtmp[:, t * 128:(t + 1) * 128], ident_r[:]
                        )
                        nc.vector.tensor_copy(
                            v_sb[:, blk * 256 + hh * 128:blk * 256 + hh * 128 + 128],
                            tp[:],
                        )

        # ---- phase 2+3: attention + output projection, per query group ----
        with ExitStack() as ph2:
            p2c = ph2.enter_context(tc.tile_pool(name="p2c", bufs=1))
            p2w = ph2.enter_context(tc.tile_pool(name="p2w", bufs=2))
            psim = ph2.enter_context(tc.tile_pool(name="psim", bufs=3, space="PSUM"))
            ppvz = ph2.enter_context(tc.tile_pool(name="ppvz", bufs=3, space="PSUM"))
            pbcop = ph2.enter_context(tc.tile_pool(name="pbcop", bufs=2, space="PSUM"))

            # wo: [128, hh*D + n]
            wo_r = p2c.tile([128, 2 * D], F32R, tag="wo", name="wor")
            for hh in range(2):
                wst = p2w.tile([128, D], F32, tag="wost", bufs=2, name="wost")
                nc.sync.dma_start(wst[:], wo_d.ap()[hh * 128:(hh + 1) * 128, :])
                nc.vector.tensor_copy(wo_r[:, hh * D:(hh + 1) * D], wst[:])

            # diagonal-block moving widths/offsets (fp32r needs free >= 256)
            DW = (512, 384, 256, 256)
            DO = (0, 128, 256, 256)
            for g in range(NG):
                gsl = slice(g * 512, (g + 1) * 512)
                nkb = 4 * (g + 1)
                nz = 2 * g + 4  # Z matmuls: 2g fold-pairs + 4 diagonal
                outT = []
                for hh in range(2):
                    pv_ps = ppvz.tile([128, 512], F32, tag="pvz", bufs=3, name="pvps")
                    z_ps = ppvz.tile([1, 512], F32, tag="pvz", bufs=3, name="zps")
                    zi = 0
                    pending = []  # non-diagonal probs awaiting fold partners
                    js = list(range(nkb))
                    for ji, j in enumerate(js):
                        p = j - 4 * g
                        diag = p >= 0
                        o, w = (DO[p], DW[p]) if diag else (0, 512)
                        sim_ps = psim.tile(
                            [128, 512], F32, tag="sim", bufs=3, name="simps"
                        )
                        nc.tensor.matmul(
                            sim_ps[:, 0:w],
                            kT[hh][:, j * 128:(j + 1) * 128],
                            qT[hh][:, g * 512 + o:(g + 1) * 512],
                            start=True,
                            stop=True,
                        )
                        probs = p2w.tile(
                            [128, 512], F32R, tag="probs", bufs=6, name="probs"
                        )
                        nc.scalar.activation(
                            probs[:, 0:w], sim_ps[:, 0:w], Exp, scale=SCALE
                        )
                        if diag:
                            # causal mask: keep iff (o+col) - part - 128p >= 0
                            nc.gpsimd.affine_select(
                                probs[:, 0:w], probs[:, 0:w],
                                pattern=[[1, w]],
                                compare_op=mybir.AluOpType.is_ge,
                                fill=0.0,
                                base=o - 128 * p,
                                channel_multiplier=-1,
                            )
                            nc.tensor.matmul(
                                z_ps[:, o:512], onescol_r[:], probs[:, 0:w],
                                start=(zi == 0), stop=(zi == nz - 1),
                                skip_group_check=True,
                            )
                            zi += 1
                        else:
                            # fold two full-width prob tiles on the idle
                            # GPSIMD engine; one ones-matmul per pair
                            pending.append(probs)
                            if len(pending) == 2:
                                zf = p2w.tile([128, 512], F32R, tag="zfold",
                                              bufs=4, name="zf")
                                nc.vector.tensor_add(
                                    zf[:], pending[0][:], pending[1][:]
                                )
                                nc.tensor.matmul(
                                    z_ps[:], onescol_r[:], zf[:],
                                    start=(zi == 0), stop=(zi == nz - 1),
                                    skip_group_check=True,
                                )
                                zi += 1
                                pending = []
                        nc.tensor.matmul(
                            pv_ps[:, o:512],
                            v_sb[:, j * 256 + hh * 128:j * 256 + hh * 128 + 128],
                            probs[:, 0:w],
                            start=(ji == 0), stop=(ji == nkb - 1),
                            skip_group_check=True,
                        )
                    assert not pending and zi == nz
                    recip = p2w.tile([1, 512], F32R, tag="recip", bufs=2, name="recip")
                    with nc.allow_low_precision(reason="fp32r rounding of 1/Z"):
                        nc.vector.reciprocal(recip[:], z_ps[:])
                    bc_ps = pbcop.tile([128, 512], F32, tag="bcop", bufs=2, name="bcps")
                    nc.tensor.matmul(
                        bc_ps[:], onesrow_r[:], recip[:], start=True, stop=True
                    )
                    bc_sb = p2w.tile([128, 512], F32, tag="bc", bufs=2, name="bcsb")
                    nc.vector.tensor_copy(bc_sb[:], bc_ps[:])
                    ot = p2w.tile([128, 512], F32R, tag="outT", bufs=6, name="outT")
                    nc.vector.tensor_mul(ot[:], pv_ps[:], bc_sb[:])
                    outT.append(ot)
                last = g == NG - 1
                for t in range(4):
                    osb = p2w.tile([128, D], F32, tag="osb", bufs=3, name="osb")
                    for n in range(4):
                        op_ps = pbcop.tile(
                            [128, 512], F32, tag="bcop", bufs=2, name="opps"
                        )
                        for hh in range(2):
                            nc.tensor.matmul(
                                op_ps[:],
                                outT[hh][:, t * 128:(t + 1) * 128],
                                wo_r[:, hh * D + n * 512:hh * D + (n + 1) * 512],
                                start=(hh == 0),
                                stop=(hh == 1),
                            )
                        nc.vector.tensor_copy(osb[:, n * 512:(n + 1) * 512], op_ps[:])
                        if last:
                            # final group sits in the kernel-exit drain shadow:
                            # ship each chunk as soon as it's copied
                            nc.sync.dma_start(
                                out_d.ap()[g * 512 + t * 128:
                                           g * 512 + (t + 1) * 128,
                                           n * 512:(n + 1) * 512],
                                osb[:, n * 512:(n + 1) * 512],
                            )
                    if not last:
                        nc.sync.dma_start(
                            out_d.ap()[g * 512 + t * 128:g * 512 + (t + 1) * 128, :],
                            osb[:],
                        )

    nc.compile()
    return nc


def _host_tables(S: int):
    """cos/sin tables, rotate-half permutation, identity, masks, ones."""
    inv = 1.0 / (ROPE_BASE ** (np.arange(0, DH, 2, dtype=np.float64) / DH))
    t = np.arange(S, dtype=np.float64)
    fr = np.outer(t, inv)  # [S, 64]
    cos = np.repeat(np.cos(fr), 2, axis=1)  # [S, DH]
    sin = np.repeat(np.sin(fr), 2, axis=1)
    cs = np.concatenate([cos.T, sin.T], axis=1).astype(np.float32)  # [128, 2S]

    PT = np.zeros((DH, DH), np.float32)
    for m in range(DH // 2):
        # rotate_half: out[2m] = -in[2m+1], out[2m+1] = in[2m]
        PT[2 * m + 1, 2 * m] = -1.0
        PT[2 * m, 2 * m + 1] = 1.0
    consts = np.zeros((128, 257), np.float32)
    consts[:, 0:128] = PT
    consts[:, 128:256] = np.eye(128, dtype=np.float32)
    consts[:, 256] = 1.0

    onesrow = np.ones((1, 128), np.float32)
    return cs, consts, onesrow


def kernel(x, mask, wq, wk, wv, wo):
    x = np.ascontiguousarray(np.asarray(x, dtype=np.float32))
    wq = np.asarray(wq, dtype=np.float32)
    wk = np.asarray(wk, dtype=np.float32)
    wv = np.asarray(wv, dtype=np.float32)
    wo = np.asarray(wo, dtype=np.float32)
    S = x.shape[0]

    if S not in _BUILD_CACHE:
        _BUILD_CACHE[S] = _build(S)
    nc = _BUILD_CACHE[S]

    cs, consts, onesrow = _host_tables(S)
    xT = np.ascontiguousarray(x.T)

    in_maps = []
    for c in range(NCORES):
        hsl = slice(c * HPC * DH, (c + 1) * HPC * DH)  # this core's 256 rows
        wqT = wq[hsl].T.reshape(D, 2, DH)
        wkT = wk[hsl].T.reshape(D, 2, DH)
        wvT = wv[hsl].T.reshape(D, 2, DH)
        # [D, 768]: cols jt*128.., jt=(q0,q1,k0,k1,v0,v1)
        wqkvT = np.concatenate(
            [wqT[:, 0], wqT[:, 1], wkT[:, 0], wkT[:, 1], wvT[:, 0], wvT[:, 1]],
            axis=1,
        )
        woT = np.ascontiguousarray(wo[:, hsl].T)  # [256, D]
        in_maps.append(
            {
                "xT": xT,
                "wqkvT": np.ascontiguousarray(wqkvT),
                "woT": woT,
                "cs": cs,
                "consts": consts,
                "onesrow": onesrow,
            }
        )

    res = run_bass_kernel_spmd(
        nc, in_maps, core_ids=list(range(NCORES)), trace=TRACE
    )
    global LAST_RESULT
    LAST_RESULT = res
    out = np.zeros((S, D), np.float32)
    for r in res.results:
        out += r["outp"]
    return out



# revision 4
# speedup vs baseline: 1.0336x; 1.0336x over previous
"""Trainium2 Bass kernel for nn_Attention_48000554500172 (v2).

16-head causal attention with RoPE (S=4096, D=2048, H=16, DH=128), sharded
over heads across 8 NeuronCores (2 heads/core, tensor parallel). Each core
computes its 2 heads and a partial [S, D] output projection in bf16; the
host upcasts and sums the 8 partials (the all-reduce).

Key design points (vs the 452us v1):
- Mixed-dtype matmuls tuned to the cost model: MOVING operands are f32r
  (>=256 free) wherever possible because a 2-byte moving operand makes the
  compiler emit an InstLdweights per matmul (~38ns of PE sequencer each).
  Stationary operands are bf16 (dtype irrelevant for cost/ldweights).
  f32r inputs (x, wv, wo) are DMA'd straight from HBM - no rounding copies.
- V is projected directly into [s, dh] layout (lhsT = x chunk), killing
  the per-block PE transposes of v1.
- Scores are computed transposed (keys on partitions) in PAIRED 2-bank
  PSUM tiles [128, 1024] (2 key blocks x 512 queries); one wide exp per
  non-diag pair. Causality for the 2 diagonal pairs is one affine_select
  each, whose 2-level iota pattern also zeroes the inter-region junk.
- Softmax denominators: probs pairs are summed into an f32r zacc on DVE
  (serial chain hidden under the exp pipeline), then ONE ones-matrix
  matmul per (g,hh) broadcasts Z across 128 partitions in PSUM: ~8k PE
  rows total vs ~88k for v1's per-block ones-matmuls + bc broadcast.
- Projection and attention are emission-interleaved (attn group g with
  projection slice g+2) so attention's sim->exp->pv latency chains are
  filled with projection matmuls; attention groups 6,7 run in a second
  PSUM scope with a deeper sim pipeline once projection banks free up.
"""
import math
import numpy as np
import ml_dtypes
from contextlib import ExitStack

import concourse.bass as bass
import concourse.tile as tile
from concourse import bacc, mybir
from concourse.bass_utils import run_bass_kernel_spmd

D, H, DH = 2048, 16, 128
NCORES = 8
HPC = H // NCORES  # 2 heads per core
ROPE_BASE = 10000.0
SCALE = 1.0 / math.sqrt(DH)
F32 = mybir.dt.float32
F32R = mybir.dt.float32r
BF16 = mybir.dt.bfloat16
Exp = mybir.ActivationFunctionType.Exp
BF = ml_dtypes.bfloat16

_BUILD_CACHE: dict = {}
TRACE = False
LAST_RESULT = None


def _interleave(a, b):
    """Merge two thunk lists proportionally."""
    out = []
    ia = ib = 0
    while ia < len(a) or ib < len(b):
        fa = ia / len(a) if a else 1.0
        fb = ib / len(b) if b else 1.0
        if ib >= len(b) or (ia < len(a) and fa <= fb):
            out.append(a[ia]); ia += 1
        else:
            out.append(b[ib]); ib += 1
    return out


def _build(S: int):
    assert S % 512 == 0
    ND = D // 128      # 16 contraction chunks
    NSUB = S // 256    # projection subslices
    NG = S // 512      # attention query groups
    NB = S // 128      # key blocks

    nc = bacc.Bacc("TRN2", target_bir_lowering=False, debug=False)

    xT_d = nc.dram_tensor("xT", [D, S], F32R, kind="ExternalInput")
    wqk_d = nc.dram_tensor("wqk", [128, ND * 512], BF16, kind="ExternalInput")
    wv_d = nc.dram_tensor("wv", [128, ND * 256], F32R, kind="ExternalInput")
    wo_d = nc.dram_tensor("wo", [128, 2 * D], F32R, kind="ExternalInput")
    cs_d = nc.dram_tensor("cs", [128, NSUB * 512], BF16, kind="ExternalInput")
    consts_d = nc.dram_tensor("consts", [128, 2304], BF16, kind="ExternalInput")
    out_d = nc.dram_tensor("outp", [S, D], BF16, kind="ExternalOutput")

    with tile.TileContext(nc) as tc, ExitStack() as ctx:
        persist = ctx.enter_context(tc.tile_pool(name="persist", bufs=1))
        work = ctx.enter_context(tc.tile_pool(name="work", bufs=2))

        # ---- persistent SBUF ----
        qT = persist.tile([128, 2 * S], BF16, tag="qT", name="qT")
        kT = persist.tile([128, 2 * S], BF16, tag="kT", name="kT")
        v_sb = persist.tile([128, NB * 256], BF16, tag="v", name="v_sb")
        wqk_sb = persist.tile([128, ND * 512], BF16, tag="wqk", name="wqk_sb")
        wv_sb = persist.tile([128, ND * 256], F32R, tag="wv", name="wv_sb")
        wo_sb = persist.tile([128, 2 * D], F32R, tag="wo", name="wo_sb")
        cs_sb = persist.tile([128, NSUB * 512], BF16, tag="cs", name="cs_sb")
        consts_sb = persist.tile([128, 2304], BF16, tag="cst", name="consts_sb")
        ones_r = persist.tile([128, 128], F32R, tag="ones", name="ones_r")

        xsub_tiles = {}

        def xsub_tile(sub):
            t = work.tile([128, ND * 256], F32R, tag="xsub", bufs=3,
                          name=f"xs{sub}")
            xsub_tiles[sub] = t
            return t

        def dma_xsub(sub, chunks=1, queue=None):
            eng = queue if queue is not None else nc.sync
            t = xsub_tile(sub)
            src = xT_d.ap()[:, sub * 256:(sub + 1) * 256]
            src3 = src.rearrange("(d p) c -> p d c", p=128)
            dst3 = t[:].rearrange("p (d c) -> p d c", d=ND)
            if chunks == 1:
                eng.dma_start(dst3, src3)
            else:
                dper = ND // chunks
                for c in range(chunks):
                    eng.dma_start(
                        dst3[:, c * dper:(c + 1) * dper, :],
                        src3[:, c * dper:(c + 1) * dper, :],
                    )

        def dma_cs(sub):
            nc.sync.dma_start(
                cs_sb[:, sub * 512:(sub + 1) * 512],
                cs_d.ap()[:, sub * 512:(sub + 1) * 512],
            )

        # ---- prologue DMAs (one serial DMA resource; ordered so the
        # first projection matmuls are fed earliest) ----
        WQC = ND * 512 // 4
        WVC = ND * 256 // 4
        nc.sync.dma_start(wqk_sb[:, 0:WQC], wqk_d.ap()[:, 0:WQC])
        dma_xsub(0, chunks=4)
        nc.sync.dma_start(consts_sb[:], consts_d.ap())
        dma_cs(0)
        for c in range(1, 4):
            nc.sync.dma_start(wqk_sb[:, c * WQC:(c + 1) * WQC],
                              wqk_d.ap()[:, c * WQC:(c + 1) * WQC])
        dma_cs(1)
        for c in range(4):
            nc.sync.dma_start(wv_sb[:, c * WVC:(c + 1) * WVC],
                              wv_d.ap()[:, c * WVC:(c + 1) * WVC])
        dma_xsub(1, chunks=2)
        for sub in range(2, 4):
            dma_cs(sub)
        dma_xsub(2)
        for c in range(4):
            q = 2 * D // 4
            nc.sync.dma_start(wo_sb[:, c * q:(c + 1) * q],
                              wo_d.ap()[:, c * q:(c + 1) * q])
        for sub in range(4, NSUB):
            dma_cs(sub)
        with nc.allow_low_precision(reason="f32r ones for Z broadcast matmul"):
            nc.vector.tensor_copy(ones_r[:], consts_sb[:, 128:256])

        qT2 = qT[:].rearrange("p (h s) -> p h s", h=2)
        kT2 = kT[:].rearrange("p (h s) -> p h s", h=2)

        # ================= emission thunk generators =================

        def proj_pieces(sub, prefetch):
            """Projection of subslice `sub` (256 tokens): q,k,v + rope."""
            pieces = []
            xs = xsub_tiles[sub]

            def qk_half(kind, dlo, dhi, acc):
                # acc pair regions: [h0 256 | h1 256]; one bank.
                for d in range(dlo, dhi):
                    for h in range(2):
                        nc.tensor.matmul(
                            acc[:, h * 256:(h + 1) * 256],
                            wqk_sb[:, d * 512 + (2 * kind + h) * 128:
                                   d * 512 + (2 * kind + h) * 128 + 128],
                            xs[:, d * 256:(d + 1) * 256],
                            start=(d == 0 and h == 0),
                            stop=(d == ND - 1 and h == 1),
                            skip_group_check=True,
                        )

            def v_half(dlo, dhi, acc):
                # acc regions: [blk0 (h0|h1) | blk1 (h0|h1)]; lhsT = x chunk.
                for d in range(dlo, dhi):
                    for b in range(2):
                        nc.tensor.matmul(
                            acc[:, b * 256:(b + 1) * 256],
                            xs[:, d * 256 + b * 128:d * 256 + b * 128 + 128],
                            wv_sb[:, d * 256:(d + 1) * 256],
                            start=(d == 0 and b == 0),
                            stop=(d == ND - 1 and b == 1),
                            skip_group_check=True,
                        )

            state = {}

            def pf():
                if prefetch is not None and prefetch < NSUB:
                    dma_xsub(prefetch)
                state["qacc"] = pqkv.tile([128, 512], F32, tag="qkv", bufs=2,
                                          name="qacc")
                qk_half(0, 0, 8, state["qacc"])

            def tin_of(which):
                def f():
                    t = work.tile([128, 512], F32R, tag="tin", bufs=2,
                                  name="tin")
                    with nc.allow_low_precision(reason="f32r rope input"):
                        nc.gpsimd.tensor_copy(t[:], state[which][:])
                    state["tin_" + which] = t
                return f

            def rope_of(which, dstT2):
                def f():
                    tin = state["tin_" + which]
                    p_ps = pmisc.tile([128, 512], F32, tag="misc", bufs=2,
                                      name="pps")
                    nc.tensor.matmul(p_ps[:], consts_sb[:, 0:128], tin[:],
                                     start=True, stop=True)
                    cos = cs_sb[:, sub * 512:sub * 512 + 256]
                    sin = cs_sb[:, sub * 512 + 256:sub * 512 + 512]
                    with nc.allow_low_precision(reason="f32r rope products"):
                        t1 = work.tile([128, 512], F32R, tag="t1", bufs=1,
                                       name="t1")
                        nc.vector.tensor_mul(t1[:, 0:256], tin[:, 0:256], cos)
                        nc.vector.tensor_mul(t1[:, 256:512], tin[:, 256:512],
                                             cos)
                        t2 = work.tile([128, 512], F32R, tag="t2", bufs=2,
                                       name="t2")
                        nc.gpsimd.tensor_mul(t2[:, 0:256], p_ps[:, 0:256], sin)
                        nc.gpsimd.tensor_mul(t2[:, 256:512], p_ps[:, 256:512],
                                             sin)
                    dst = dstT2[:, :, sub * 256:(sub + 1) * 256]
                    nc.vector.tensor_add(
                        dst,
                        t1[:].rearrange("p (h s) -> p h s", h=2),
                        t2[:].rearrange("p (h s) -> p h s", h=2),
                    )
                return f

            def k1():
                state["kacc"] = pqkv.tile([128, 512], F32, tag="qkv", bufs=2,
                                          name="kacc")
                qk_half(1, 0, 8, state["kacc"])

            def v1():
                state["vacc"] = pqkv.tile([128, 512], F32, tag="qkv", bufs=2,
                                          name="vacc")
                v_half(0, 8, state["vacc"])

            pieces.append(pf)
            pieces.append(lambda: qk_half(0, 8, ND, state["qacc"]))
            pieces.append(tin_of("qacc"))
            pieces.append(k1)
            pieces.append(lambda: qk_half(1, 8, ND, state["kacc"]))
            pieces.append(tin_of("kacc"))
            pieces.append(v1)
            pieces.append(lambda: v_half(8, ND, state["vacc"]))
            pieces.append(rope_of("qacc", qT2))
            pieces.append(rope_of("kacc", kT2))
            pieces.append(lambda: nc.gpsimd.tensor_copy(
                v_sb[:, sub * 512:(sub + 1) * 512], state["vacc"][:]))
            return pieces

        def attn_pieces(g, simpool, simbufs, pvpool, zpool, ztag,
                        zbufs, chunked_out, split_exp=False,
                        drain_engines=("gpsimd", "dve")):
            """Attention group g (512 queries), both heads + out projection.

            The two heads' pair thunks are interleaved so each head's
            exp/mask/Z latency is hidden under the other head's matmuls.
            chunked_out: ship each 512-col output chunk as its own DMA (for
            the final groups, to hide the store in the kernel drain shadow).
            """
            npair = 2 * (g + 1)
            gq = g * 512
            ots = {}
            states = {0: {}, 1: {}}

            def mk_pair(hh, pi):
                st = states[hh]

                def f():
                    if pi == 0:
                        st["pv"] = pvpool.tile([128, 512], F32, tag="pv",
                                               bufs=2, name="pv")
                    diag = pi >= npair - 2
                    pair = simpool.tile([128, 1024], F32, tag="sim",
                                        bufs=simbufs, name="sim")
                    probs = work.tile([128, 1024], BF16, tag="probs",
                                      bufs=5, name="probs")
                    jA, jB = 2 * pi, 2 * pi + 1
                    if not diag:
                        nc.tensor.matmul(
                            pair[:, 0:512],
                            kT[:, hh * S + jA * 128:hh * S + jA * 128 + 128],
                            qT[:, hh * S + gq:hh * S + gq + 512],
                            start=True, stop=True, skip_group_check=True)
                        nc.tensor.matmul(
                            pair[:, 512:1024],
                            kT[:, hh * S + jB * 128:hh * S + jB * 128 + 128],
                            qT[:, hh * S + gq:hh * S + gq + 512],
                            start=True, stop=True, skip_group_check=True)
                        if split_exp:
                            # halves as soon as each sim lands: shorter
                            # WAR latency for the 1-deep scope-1 pipeline
                            nc.scalar.activation(probs[:, 0:512],
                                                 pair[:, 0:512], Exp,
                                                 scale=SCALE)
                            nc.scalar.activation(probs[:, 512:1024],
                                                 pair[:, 512:1024], Exp,
                                                 scale=SCALE)
                        else:
                            nc.scalar.activation(probs[:], pair[:], Exp,
                                                 scale=SCALE)
                        pvA = (0, 0)   # (out offset, probs offset)
                        pvB = (0, 512)
                    else:
                        d = pi - (npair - 2)  # 0 or 1
                        oA, oB = (0, 128) if d == 0 else (256, 384)
                        # zero the never-exp'd junk early (off critical path)
                        if oA > 0:
                            nc.gpsimd.memset(probs[:, 0:oA], 0.0)
                        nc.gpsimd.memset(probs[:, 512:512 + oB], 0.0)
                        nc.tensor.matmul(
                            pair[:, oA:512],
                            kT[:, hh * S + jA * 128:hh * S + jA * 128 + 128],
                            qT[:, hh * S + gq + oA:hh * S + gq + 512],
                            start=True, stop=True, skip_group_check=True)
                        nc.tensor.matmul(
                            pair[:, 512 + oB:1024],
                            kT[:, hh * S + jB * 128:hh * S + jB * 128 + 128],
                            qT[:, hh * S + gq + oB:hh * S + gq + 512],
                            start=True, stop=True, skip_group_check=True)
                        nc.scalar.activation(probs[:, oA:512],
                                             pair[:, oA:512], Exp,
                                             scale=SCALE)
                        nc.scalar.activation(probs[:, 512 + oB:1024],
                                             pair[:, 512 + oB:1024],
                                             Exp, scale=SCALE)
                        # causal mask + junk zeroing via precomputed mask
                        # tiles (DVE 2x); per-half so each PV matmul waits
                        # only its own half's mask
                        nc.vector.tensor_mul(
                            probs[:, 0:512], probs[:, 0:512],
                            consts_sb[:, 256 + d * 1024:256 + d * 1024 + 512])
                        nc.vector.tensor_mul(
                            probs[:, 512:1024], probs[:, 512:1024],
                            consts_sb[:, 768 + d * 1024:768 + d * 1024 + 512])
                        pvA = (oA, oA)
                        pvB = (oB, 512 + oB)
                    # Z: bf16 halves-add (DVE 2x), then f32r accumulate.
                    # The last pair skips the chain; its halves-sum feeds a
                    # second accumulating Z-matmul directly so the in-order
                    # PE never waits for the chain tail.
                    hs = work.tile([128, 512], BF16, tag="hs", bufs=2,
                                   name="hs")
                    nc.vector.tensor_add(hs[:], probs[:, 0:512],
                                         probs[:, 512:1024])
                    with nc.allow_low_precision(reason="f32r Z accum"):
                        if pi == 0:
                            st["zacc"] = work.tile([128, 512], F32R,
                                                   tag="zacc", bufs=2,
                                                   name="zacc")
                            nc.vector.tensor_copy(st["zacc"][:], hs[:])
                        elif pi < npair - 1:
                            nc.vector.tensor_add(st["zacc"][:],
                                                 st["zacc"][:], hs[:])
                        else:
                            st["hs_last"] = hs
                    nc.tensor.matmul(
                        st["pv"][:, pvA[0]:512],
                        v_sb[:, jA * 256 + hh * 128:jA * 256 + hh * 128 + 128],
                        probs[:, pvA[1]:pvA[1] + 512 - pvA[0]],
                        start=(pi == 0), stop=False,
                        skip_group_check=True)
                    nc.tensor.matmul(
                        st["pv"][:, pvB[0]:512],
                        v_sb[:, jB * 256 + hh * 128:jB * 256 + hh * 128 + 128],
                        probs[:, pvB[1]:pvB[1] + 512 - pvB[0]],
                        start=False, stop=(pi == npair - 1),
                        skip_group_check=True)
                return f

            def mk_ztail(hh):
                st = states[hh]

                def f():
                    zbc = zpool.tile([128, 512], F32, tag=ztag, bufs=zbufs,
                                     name="zbc")
                    nc.tensor.matmul(zbc[:], ones_r[:], st["zacc"][:],
                                     start=True, stop=False)
                    nc.tensor.matmul(zbc[:], ones_r[:], st["hs_last"][:],
                                     start=False, stop=True)
                    recip = work.tile([128, 512], F32, tag="recip", bufs=2,
                                      name="recip")
                    nc.vector.reciprocal(recip[:], zbc[:])
                    ot = work.tile([128, 512], BF16, tag="ot", bufs=4,
                                   name="ot")
                    nc.vector.tensor_mul(ot[:], st["pv"][:], recip[:])
                    ots[hh] = ot
                return f

            pieces = []
            for pi in range(npair):
                pieces.append(mk_pair(0, pi))
                pieces.append(mk_pair(1, pi))
            pieces.append(mk_ztail(0))
            pieces.append(mk_ztail(1))

            # out projection: 4 token-blocks x 4 d-chunks
            osb_state = {}

            def mk_op(t, n, oppool, opbufs, optag):
                def f():
                    if n == 0:
                        osb_state[t] = work.tile([128, D], BF16, tag="osb",
                                                 bufs=2, name="osb")
                    osb = osb_state[t]
                    op = oppool.tile([128, 512], F32, tag=optag, bufs=opbufs,
                                     name="op")
                    for hh in range(2):
                        nc.tensor.matmul(
                            op[:],
                            ots[hh][:, t * 128:(t + 1) * 128],
                            wo_sb[:, hh * D + n * 512:hh * D + (n + 1) * 512],
                            start=(hh == 0), stop=(hh == 1))
                    deng = drain_engines[(t * 4 + n) % len(drain_engines)]
                    if deng == "act":
                        nc.scalar.copy(osb[:, n * 512:(n + 1) * 512], op[:])
                    elif deng == "dve":
                        nc.vector.tensor_copy(osb[:, n * 512:(n + 1) * 512],
                                              op[:])
                    else:
                        nc.gpsimd.tensor_copy(osb[:, n * 512:(n + 1) * 512],
                                              op[:])
                    if chunked_out:
                        nc.sync.dma_start(
                            out_d.ap()[g * 512 + t * 128:
                                       g * 512 + (t + 1) * 128,
                                       n * 512:(n + 1) * 512],
                            osb[:, n * 512:(n + 1) * 512])
                    elif n == 3:
                        nc.sync.dma_start(
                            out_d.ap()[g * 512 + t * 128:
                                       g * 512 + (t + 1) * 128, :],
                            osb[:])
                return f

            def op_factory(oppool, opbufs, optag):
                return [mk_op(t, n, oppool, opbufs, optag)
                        for t in range(4) for n in range(4)]
            return pieces, op_factory

        # ================= schedule =================
        with ExitStack() as s1:
            pqkv = s1.enter_context(
                tc.tile_pool(name="pqkv", bufs=2, space="PSUM"))
            pmisc = s1.enter_context(
                tc.tile_pool(name="pmisc", bufs=2, space="PSUM"))
            psim = s1.enter_context(
                tc.tile_pool(name="psim", bufs=1, space="PSUM"))
            ppv = s1.enter_context(
                tc.tile_pool(name="ppv", bufs=2, space="PSUM"))

            # x0..x2 are loaded by the prologue; prefetch distance 2 with
            # bufs=3 (prefetching sub+3 would race the current sub's buffer)
            for p in proj_pieces(0, prefetch=None):
                p()
            for p in proj_pieces(1, prefetch=3):
                p()
            for p in proj_pieces(2, prefetch=4):
                p()
            for p in proj_pieces(3, prefetch=5):
                p()
            # brackets: attn(g) + proj slice g+2 (subs 2g+4, 2g+5), g=0..5.
            # Each group's out-projection pieces are pure PE work and are
            # deferred into the NEXT bracket as chain filler.
            pending_mkops = None
            for g in range(NG - 2):
                ap, mkops = attn_pieces(g, psim, 1, ppv, pmisc, "misc", 2,
                                        chunked_out=False, split_exp=True)
                pp = proj_pieces(2 * g + 4, prefetch=2 * g + 6)
                pp += proj_pieces(2 * g + 5, prefetch=2 * g + 7)
                if pending_mkops is not None:
                    pp = pp + pending_mkops(pmisc, 2, "misc")
                for p in _interleave(ap, pp):
                    p()
                pending_mkops = mkops

        with ExitStack() as s2:
            psim2 = s2.enter_context(
                tc.tile_pool(name="psim2", bufs=2, space="PSUM"))
            ppv2 = s2.enter_context(
                tc.tile_pool(name="ppv2", bufs=2, space="PSUM"))
            popz = s2.enter_context(
                tc.tile_pool(name="popz", bufs=2, space="PSUM"))
            a6, mkops6 = attn_pieces(NG - 2, psim2, 2, ppv2, popz, "opz", 2,
                                     chunked_out=True)
            a7, mkops7 = attn_pieces(NG - 1, psim2, 2, ppv2, popz, "opz", 2,
                                     chunked_out=True)
            # attn(6) with ops(5) as filler, then attn(7) with ops(6).
            # (Fully interleaving the two groups deadlocks: 4 live PV
            # accumulators vs 2 banks.)
            ops5 = pending_mkops(popz, 2, "opz") if pending_mkops else []
            for p in _interleave(a6, ops5):
                p()
            for p in _interleave(a7, mkops6(popz, 2, "opz")):
                p()
        with ExitStack() as s3:
            # group 7's out-projection alone at the very end: give it 4
            # PSUM banks so the matmul/drain rotation never stalls
            pop3 = s3.enter_context(
                tc.tile_pool(name="pop3", bufs=4, space="PSUM"))
            for p in mkops7(pop3, 4, "op3"):
                p()

    nc.dbg_tiles = {"qT": qT, "kT": kT, "v_sb": v_sb}
    nc.compile()
    return nc


def _host_tables(S: int):
    NSUB = S // 256
    inv = 1.0 / (ROPE_BASE ** (np.arange(0, DH, 2, dtype=np.float64) / DH))
    t = np.arange(S, dtype=np.float64)
    fr = np.outer(t, inv)  # [S, 64]
    cos = np.repeat(np.cos(fr), 2, axis=1).T  # [128, S]
    sin = np.repeat(np.sin(fr), 2, axis=1).T
    cs = np.zeros((128, NSUB * 512), np.float32)
    for sub in range(NSUB):
        cs[:, sub * 512:sub * 512 + 256] = cos[:, sub * 256:(sub + 1) * 256]
        cs[:, sub * 512 + 256:sub * 512 + 512] = sin[:, sub * 256:(sub + 1) * 256]

    PT = np.zeros((DH, DH), np.float32)
    for m in range(DH // 2):
        PT[2 * m + 1, 2 * m] = -1.0
        PT[2 * m, 2 * m + 1] = 1.0
    consts = np.zeros((128, 2304), np.float32)
    consts[:, 0:128] = PT
    consts[:, 128:256] = 1.0
    # causal masks for the two diagonal pair tiles: regions [0:512] and
    # [512:1024] hold key blocks (4g+2d) and (4g+2d+1); keep iff
    # query_col >= key_part + 128*(2d+j)
    p = np.arange(128)[:, None]
    c = np.arange(512)[None, :]
    for d in range(2):
        m0 = (c >= p + 256 * d).astype(np.float32)
        m1 = (c >= p + 256 * d + 128).astype(np.float32)
        consts[:, 256 + d * 1024:256 + d * 1024 + 512] = m0
        consts[:, 256 + d * 1024 + 512:256 + (d + 1) * 1024] = m1
    return cs.astype(BF), consts.astype(BF)


def _host_inputs(x, wq, wk, wv, wo, S):
    """Per-core input maps."""
    ND = D // 128
    cs, consts = _host_tables(S)
    xT = np.ascontiguousarray(x.T.astype(np.float32))

    in_maps = []
    for c in range(NCORES):
        hsl = slice(c * HPC * DH, (c + 1) * HPC * DH)
        wqT = wq[hsl].T.astype(BF)  # [D, 256]
        wkT = wk[hsl].T.astype(BF)
        wvT = wv[hsl].T.astype(np.float32)
        wqk = np.zeros((128, ND * 512), BF)
        wvh = np.zeros((128, ND * 256), np.float32)
        for d in range(ND):
            wqk[:, d * 512:d * 512 + 256] = wqT[d * 128:(d + 1) * 128]
            wqk[:, d * 512 + 256:d * 512 + 512] = wkT[d * 128:(d + 1) * 128]
            wvh[:, d * 256:(d + 1) * 256] = wvT[d * 128:(d + 1) * 128]
        woT = wo[:, hsl].T.astype(np.float32)  # [256, D]
        wo_sb = np.concatenate([woT[0:128], woT[128:256]], axis=1)  # [128, 2D]
        in_maps.append({
            "xT": xT,
            "wqk": np.ascontiguousarray(wqk),
            "wv": np.ascontiguousarray(wvh),
            "wo": np.ascontiguousarray(wo_sb),
            "cs": cs,
            "consts": consts,
        })
    return in_maps


def kernel(x, mask, wq, wk, wv, wo):
    x = np.asarray(x, dtype=np.float32)
    wq = np.asarray(wq, dtype=np.float32)
    wk = np.asarray(wk, dtype=np.float32)
    wv = np.asarray(wv, dtype=np.float32)
    wo = np.asarray(wo, dtype=np.float32)
    S = x.shape[0]

    if S not in _BUILD_CACHE:
        _BUILD_CACHE[S] = _build(S)
    nc = _BUILD_CACHE[S]

    in_maps = _host_inputs(x, wq, wk, wv, wo, S)
    res = run_bass_kernel_spmd(
        nc, in_maps, core_ids=list(range(NCORES)), trace=TRACE
    )
    global LAST_RESULT
    LAST_RESULT = res
    out = np.zeros((S, D), np.float32)
    for r in res.results:
        out += r["outp"].astype(np.float32)
    return out


# revision 5
# speedup vs baseline: 1.0370x; 1.0033x over previous
"""Trainium2 Bass kernel for nn_Attention_48000554500172 (v2).

16-head causal attention with RoPE (S=4096, D=2048, H=16, DH=128), sharded
over heads across 8 NeuronCores (2 heads/core, tensor parallel). Each core
computes its 2 heads and a partial [S, D] output projection in bf16; the
host upcasts and sums the 8 partials (the all-reduce).

Key design points (vs the 452us v1):
- Mixed-dtype matmuls tuned to the cost model: MOVING operands are f32r
  (>=256 free) wherever possible because a 2-byte moving operand makes the
  compiler emit an InstLdweights per matmul (~38ns of PE sequencer each).
  Stationary operands are bf16 (dtype irrelevant for cost/ldweights).
  f32r inputs (x, wv, wo) are DMA'd straight from HBM - no rounding copies.
- V is projected directly into [s, dh] layout (lhsT = x chunk), killing
  the per-block PE transposes of v1.
- Scores are computed transposed (keys on partitions) in PAIRED 2-bank
  PSUM tiles [128, 1024] (2 key blocks x 512 queries); one wide exp per
  non-diag pair. Causality for the 2 diagonal pairs is one affine_select
  each, whose 2-level iota pattern also zeroes the inter-region junk.
- Softmax denominators: probs pairs are summed into an f32r zacc on DVE
  (serial chain hidden under the exp pipeline), then ONE ones-matrix
  matmul per (g,hh) broadcasts Z across 128 partitions in PSUM: ~8k PE
  rows total vs ~88k for v1's per-block ones-matmuls + bc broadcast.
- Projection and attention are emission-interleaved (attn group g with
  projection slice g+2) so attention's sim->exp->pv latency chains are
  filled with projection matmuls; attention groups 6,7 run in a second
  PSUM scope with a deeper sim pipeline once projection banks free up.
"""
import math
import numpy as np
import ml_dtypes
from contextlib import ExitStack

import concourse.bass as bass
import concourse.tile as tile
from concourse import bacc, mybir
from concourse.bass_utils import run_bass_kernel_spmd

D, H, DH = 2048, 16, 128
NCORES = 8
HPC = H // NCORES  # 2 heads per core
ROPE_BASE = 10000.0
SCALE = 1.0 / math.sqrt(DH)
F32 = mybir.dt.float32
F32R = mybir.dt.float32r
BF16 = mybir.dt.bfloat16
Exp = mybir.ActivationFunctionType.Exp
BF = ml_dtypes.bfloat16

_BUILD_CACHE: dict = {}
TRACE = False
LAST_RESULT = None


def _interleave(a, b):
    """Merge two thunk lists proportionally."""
    out = []
    ia = ib = 0
    while ia < len(a) or ib < len(b):
        fa = ia / len(a) if a else 1.0
        fb = ib / len(b) if b else 1.0
        if ib >= len(b) or (ia < len(a) and fa <= fb):
            out.append(a[ia]); ia += 1
        else:
            out.append(b[ib]); ib += 1
    return out


def _build(S: int):
    assert S % 512 == 0
    ND = D // 128      # 16 contraction chunks
    NSUB = S // 256    # projection subslices
    NG = S // 512      # attention query groups
    NB = S // 128      # key blocks

    nc = bacc.Bacc("TRN2", target_bir_lowering=False, debug=False)

    xT_d = nc.dram_tensor("xT", [D, S], F32R, kind="ExternalInput")
    wqk_d = nc.dram_tensor("wqk", [128, ND * 512], BF16, kind="ExternalInput")
    wv_d = nc.dram_tensor("wv", [128, ND * 256], F32R, kind="ExternalInput")
    wo_d = nc.dram_tensor("wo", [128, 2 * D], F32R, kind="ExternalInput")
    cs_d = nc.dram_tensor("cs", [128, NSUB * 512], BF16, kind="ExternalInput")
    consts_d = nc.dram_tensor("consts", [128, 2304], BF16, kind="ExternalInput")
    out_d = nc.dram_tensor("outp", [S, D], BF16, kind="ExternalOutput")

    with tile.TileContext(nc) as tc, ExitStack() as ctx:
        persist = ctx.enter_context(tc.tile_pool(name="persist", bufs=1))
        work = ctx.enter_context(tc.tile_pool(name="work", bufs=2))

        # ---- persistent SBUF ----
        qT = persist.tile([128, 2 * S], BF16, tag="qT", name="qT")
        kT = persist.tile([128, 2 * S], BF16, tag="kT", name="kT")
        v_sb = persist.tile([128, NB * 256], BF16, tag="v", name="v_sb")
        wqk_sb = persist.tile([128, ND * 512], BF16, tag="wqk", name="wqk_sb")
        wv_sb = persist.tile([128, ND * 256], F32R, tag="wv", name="wv_sb")
        wo_sb = persist.tile([128, 2 * D], F32R, tag="wo", name="wo_sb")
        cs_sb = persist.tile([128, NSUB * 512], BF16, tag="cs", name="cs_sb")
        consts_sb = persist.tile([128, 2304], BF16, tag="cst", name="consts_sb")
        ones_r = persist.tile([128, 128], F32R, tag="ones", name="ones_r")

        xsub_tiles = {}

        def xsub_tile(sub):
            t = work.tile([128, ND * 256], F32R, tag="xsub", bufs=3,
                          name=f"xs{sub}")
            xsub_tiles[sub] = t
            return t

        def dma_xsub(sub, chunks=1, queue=None):
            eng = queue if queue is not None else nc.sync
            t = xsub_tile(sub)
            src = xT_d.ap()[:, sub * 256:(sub + 1) * 256]
            src3 = src.rearrange("(d p) c -> p d c", p=128)
            dst3 = t[:].rearrange("p (d c) -> p d c", d=ND)
            if chunks == 1:
                eng.dma_start(dst3, src3)
            else:
                dper = ND // chunks
                for c in range(chunks):
                    eng.dma_start(
                        dst3[:, c * dper:(c + 1) * dper, :],
                        src3[:, c * dper:(c + 1) * dper, :],
                    )

        def dma_cs(sub):
            nc.sync.dma_start(
                cs_sb[:, sub * 512:(sub + 1) * 512],
                cs_d.ap()[:, sub * 512:(sub + 1) * 512],
            )

        # ---- prologue DMAs (one serial DMA resource; ordered so the
        # first projection matmuls are fed earliest) ----
        WQC = ND * 512 // 4
        WVC = ND * 256 // 4
        nc.sync.dma_start(wqk_sb[:, 0:WQC], wqk_d.ap()[:, 0:WQC])
        dma_xsub(0, chunks=4)
        nc.sync.dma_start(consts_sb[:], consts_d.ap())
        dma_cs(0)
        for c in range(1, 4):
            nc.sync.dma_start(wqk_sb[:, c * WQC:(c + 1) * WQC],
                              wqk_d.ap()[:, c * WQC:(c + 1) * WQC])
        dma_cs(1)
        for c in range(4):
            nc.sync.dma_start(wv_sb[:, c * WVC:(c + 1) * WVC],
                              wv_d.ap()[:, c * WVC:(c + 1) * WVC])
        dma_xsub(1, chunks=2)
        for sub in range(2, 4):
            dma_cs(sub)
        dma_xsub(2)
        for c in range(4):
            q = 2 * D // 4
            nc.sync.dma_start(wo_sb[:, c * q:(c + 1) * q],
                              wo_d.ap()[:, c * q:(c + 1) * q])
        for sub in range(4, NSUB):
            dma_cs(sub)
        with nc.allow_low_precision(reason="f32r ones for Z broadcast matmul"):
            nc.vector.tensor_copy(ones_r[:], consts_sb[:, 128:256])

        qT2 = qT[:].rearrange("p (h s) -> p h s", h=2)
        kT2 = kT[:].rearrange("p (h s) -> p h s", h=2)

        # ================= emission thunk generators =================

        def proj_pieces(sub, prefetch):
            """Projection of subslice `sub` (256 tokens): q,k,v + rope."""
            pieces = []
            xs = xsub_tiles[sub]

            def qk_half(kind, dlo, dhi, acc):
                # acc pair regions: [h0 256 | h1 256]; one bank.
                for d in range(dlo, dhi):
                    for h in range(2):
                        nc.tensor.matmul(
                            acc[:, h * 256:(h + 1) * 256],
                            wqk_sb[:, d * 512 + (2 * kind + h) * 128:
                                   d * 512 + (2 * kind + h) * 128 + 128],
                            xs[:, d * 256:(d + 1) * 256],
                            start=(d == 0 and h == 0),
                            stop=(d == ND - 1 and h == 1),
                            skip_group_check=True,
                        )

            def v_half(dlo, dhi, acc):
                # acc regions: [blk0 (h0|h1) | blk1 (h0|h1)]; lhsT = x chunk.
                for d in range(dlo, dhi):
                    for b in range(2):
                        nc.tensor.matmul(
                            acc[:, b * 256:(b + 1) * 256],
                            xs[:, d * 256 + b * 128:d * 256 + b * 128 + 128],
                            wv_sb[:, d * 256:(d + 1) * 256],
                            start=(d == 0 and b == 0),
                            stop=(d == ND - 1 and b == 1),
                            skip_group_check=True,
                        )

            state = {}

            def pf():
                if prefetch is not None and prefetch < NSUB:
                    dma_xsub(prefetch)
                state["qacc"] = pqkv.tile([128, 512], F32, tag="qkv", bufs=2,
                                          name="qacc")
                qk_half(0, 0, 8, state["qacc"])

            def tin_of(which):
                def f():
                    t = work.tile([128, 512], F32R, tag="tin", bufs=2,
                                  name="tin")
                    nc.scalar.copy(t[:], state[which][:])
                    state["tin_" + which] = t
                return f

            def rope_of(which, dstT2):
                def f():
                    tin = state["tin_" + which]
                    p_ps = pmisc.tile([128, 512], F32, tag="misc", bufs=2,
                                      name="pps")
                    nc.tensor.matmul(p_ps[:], consts_sb[:, 0:128], tin[:],
                                     start=True, stop=True)
                    cos = cs_sb[:, sub * 512:sub * 512 + 256]
                    sin = cs_sb[:, sub * 512 + 256:sub * 512 + 512]
                    with nc.allow_low_precision(reason="f32r rope products"):
                        t1 = work.tile([128, 512], F32R, tag="t1", bufs=1,
                                       name="t1")
                        nc.vector.tensor_mul(t1[:, 0:256], tin[:, 0:256], cos)
                        nc.vector.tensor_mul(t1[:, 256:512], tin[:, 256:512],
                                             cos)
                        t2 = work.tile([128, 512], F32R, tag="t2", bufs=2,
                                       name="t2")
                        nc.gpsimd.tensor_mul(t2[:, 0:256], p_ps[:, 0:256], sin)
                        nc.gpsimd.tensor_mul(t2[:, 256:512], p_ps[:, 256:512],
                                             sin)
                    dst = dstT2[:, :, sub * 256:(sub + 1) * 256]
                    nc.vector.tensor_add(
                        dst,
                        t1[:].rearrange("p (h s) -> p h s", h=2),
                        t2[:].rearrange("p (h s) -> p h s", h=2),
                    )
                return f

            def k1():
                state["kacc"] = pqkv.tile([128, 512], F32, tag="qkv", bufs=2,
                                          name="kacc")
                qk_half(1, 0, 8, state["kacc"])

            def v1():
                state["vacc"] = pqkv.tile([128, 512], F32, tag="qkv", bufs=2,
                                          name="vacc")
                v_half(0, 8, state["vacc"])

            pieces.append(pf)
            pieces.append(lambda: qk_half(0, 8, ND, state["qacc"]))
            pieces.append(tin_of("qacc"))
            pieces.append(k1)
            pieces.append(lambda: qk_half(1, 8, ND, state["kacc"]))
            pieces.append(tin_of("kacc"))
            pieces.append(v1)
            pieces.append(lambda: v_half(8, ND, state["vacc"]))
            pieces.append(rope_of("qacc", qT2))
            pieces.append(rope_of("kacc", kT2))
            pieces.append(lambda: nc.scalar.copy(
                v_sb[:, sub * 512:(sub + 1) * 512], state["vacc"][:]))
            return pieces

        def attn_pieces(g, simpool, simbufs, pvpool, zpool, ztag,
                        zbufs, chunked_out, split_exp=False,
                        drain_engines=("gpsimd", "dve")):
            """Attention group g (512 queries), both heads + out projection.

            The two heads' pair thunks are interleaved so each head's
            exp/mask/Z latency is hidden under the other head's matmuls.
            chunked_out: ship each 512-col output chunk as its own DMA (for
            the final groups, to hide the store in the kernel drain shadow).
            """
            npair = 2 * (g + 1)
            gq = g * 512
            ots = {}
            states = {0: {}, 1: {}}

            def mk_pair(hh, pi):
                st = states[hh]

                def f():
                    if pi == 0:
                        st["pv"] = pvpool.tile([128, 512], F32, tag="pv",
                                               bufs=2, name="pv")
                    diag = pi >= npair - 2
                    pair = simpool.tile([128, 1024], F32, tag="sim",
                                        bufs=simbufs, name="sim")
                    probs = work.tile([128, 1024], BF16, tag="probs",
                                      bufs=5, name="probs")
                    jA, jB = 2 * pi, 2 * pi + 1
                    if not diag:
                        nc.tensor.matmul(
                            pair[:, 0:512],
                            kT[:, hh * S + jA * 128:hh * S + jA * 128 + 128],
                            qT[:, hh * S + gq:hh * S + gq + 512],
                            start=True, stop=True, skip_group_check=True)
                        nc.tensor.matmul(
                            pair[:, 512:1024],
                            kT[:, hh * S + jB * 128:hh * S + jB * 128 + 128],
                            qT[:, hh * S + gq:hh * S + gq + 512],
                            start=True, stop=True, skip_group_check=True)
                        if split_exp:
                            # halves as soon as each sim lands: shorter
                            # WAR latency for the 1-deep scope-1 pipeline
                            nc.scalar.activation(probs[:, 0:512],
                                                 pair[:, 0:512], Exp,
                                                 scale=SCALE)
                            nc.scalar.activation(probs[:, 512:1024],
                                                 pair[:, 512:1024], Exp,
                                                 scale=SCALE)
                        else:
                            nc.scalar.activation(probs[:], pair[:], Exp,
                                                 scale=SCALE)
                        pvA = (0, 0)   # (out offset, probs offset)
                        pvB = (0, 512)
                    else:
                        d = pi - (npair - 2)  # 0 or 1
                        oA, oB = (0, 128) if d == 0 else (256, 384)
                        # zero the never-exp'd junk early (off critical path)
                        if oA > 0:
                            nc.gpsimd.memset(probs[:, 0:oA], 0.0)
                        nc.gpsimd.memset(probs[:, 512:512 + oB], 0.0)
                        nc.tensor.matmul(
                            pair[:, oA:512],
                            kT[:, hh * S + jA * 128:hh * S + jA * 128 + 128],
                            qT[:, hh * S + gq + oA:hh * S + gq + 512],
                            start=True, stop=True, skip_group_check=True)
                        nc.tensor.matmul(
                            pair[:, 512 + oB:1024],
                            kT[:, hh * S + jB * 128:hh * S + jB * 128 + 128],
                            qT[:, hh * S + gq + oB:hh * S + gq + 512],
                            start=True, stop=True, skip_group_check=True)
                        nc.scalar.activation(probs[:, oA:512],
                                             pair[:, oA:512], Exp,
                                             scale=SCALE)
                        nc.scalar.activation(probs[:, 512 + oB:1024],
                                             pair[:, 512 + oB:1024],
                                             Exp, scale=SCALE)
                        # causal mask + junk zeroing via precomputed mask
                        # tiles (DVE 2x); per-half so each PV matmul waits
                        # only its own half's mask
                        nc.vector.tensor_mul(
                            probs[:, 0:512], probs[:, 0:512],
                            consts_sb[:, 256 + d * 1024:256 + d * 1024 + 512])
                        nc.vector.tensor_mul(
                            probs[:, 512:1024], probs[:, 512:1024],
                            consts_sb[:, 768 + d * 1024:768 + d * 1024 + 512])
                        pvA = (oA, oA)
                        pvB = (oB, 512 + oB)
                    # Z: bf16 halves-add (DVE 2x), then f32r accumulate.
                    # The last pair skips the chain; its halves-sum feeds a
                    # second accumulating Z-matmul directly so the in-order
                    # PE never waits for the chain tail.
                    hs = work.tile([128, 512], BF16, tag="hs", bufs=2,
                                   name="hs")
                    nc.vector.tensor_add(hs[:], probs[:, 0:512],
                                         probs[:, 512:1024])
                    with nc.allow_low_precision(reason="f32r Z accum"):
                        if pi == 0:
                            st["zacc"] = work.tile([128, 512], F32R,
                                                   tag="zacc", bufs=2,
                                                   name="zacc")
                            nc.vector.tensor_copy(st["zacc"][:], hs[:])
                        elif pi < npair - 1:
                            nc.vector.tensor_add(st["zacc"][:],
                                                 st["zacc"][:], hs[:])
                        else:
                            st["hs_last"] = hs
                    nc.tensor.matmul(
                        st["pv"][:, pvA[0]:512],
                        v_sb[:, jA * 256 + hh * 128:jA * 256 + hh * 128 + 128],
                        probs[:, pvA[1]:pvA[1] + 512 - pvA[0]],
                        start=(pi == 0), stop=False,
                        skip_group_check=True)
                    nc.tensor.matmul(
                        st["pv"][:, pvB[0]:512],
                        v_sb[:, jB * 256 + hh * 128:jB * 256 + hh * 128 + 128],
                        probs[:, pvB[1]:pvB[1] + 512 - pvB[0]],
                        start=False, stop=(pi == npair - 1),
                        skip_group_check=True)
                return f

            def mk_ztail(hh):
                st = states[hh]

                def f():
                    zbc = zpool.tile([128, 512], F32, tag=ztag, bufs=zbufs,
                                     name="zbc")
                    nc.tensor.matmul(zbc[:], ones_r[:], st["zacc"][:],
                                     start=True, stop=False)
                    nc.tensor.matmul(zbc[:], ones_r[:], st["hs_last"][:],
                                     start=False, stop=True)
                    recip = work.tile([128, 512], F32, tag="recip", bufs=2,
                                      name="recip")
                    nc.vector.reciprocal(recip[:], zbc[:])
                    ot = work.tile([128, 512], BF16, tag="ot", bufs=4,
                                   name="ot")
                    nc.vector.tensor_mul(ot[:], st["pv"][:], recip[:])
                    ots[hh] = ot
                return f

            pieces = []
            for pi in range(npair):
                pieces.append(mk_pair(0, pi))
                pieces.append(mk_pair(1, pi))
            pieces.append(mk_ztail(0))
            pieces.append(mk_ztail(1))

            # out projection: 4 token-blocks x 4 d-chunks
            osb_state = {}

            def mk_op(t, n, oppool, opbufs, optag):
                def f():
                    if n == 0:
                        osb_state[t] = work.tile([128, D], BF16, tag="osb",
                                                 bufs=2, name="osb")
                    osb = osb_state[t]
                    op = oppool.tile([128, 512], F32, tag=optag, bufs=opbufs,
                                     name="op")
                    for hh in range(2):
                        nc.tensor.matmul(
                            op[:],
                            ots[hh][:, t * 128:(t + 1) * 128],
                            wo_sb[:, hh * D + n * 512:hh * D + (n + 1) * 512],
                            start=(hh == 0), stop=(hh == 1))
                    if (t * 4 + n) % 2 == 0:
                        nc.scalar.copy(osb[:, n * 512:(n + 1) * 512], op[:])
                    else:
                        nc.vector.tensor_copy(osb[:, n * 512:(n + 1) * 512],
                                              op[:])
                    if chunked_out:
                        nc.sync.dma_start(
                            out_d.ap()[g * 512 + t * 128:
                                       g * 512 + (t + 1) * 128,
                                       n * 512:(n + 1) * 512],
                            osb[:, n * 512:(n + 1) * 512])
                    elif n == 3:
                        nc.sync.dma_start(
                            out_d.ap()[g * 512 + t * 128:
                                       g * 512 + (t + 1) * 128, :],
                            osb[:])
                return f

            def op_factory(oppool, opbufs, optag):
                return [mk_op(t, n, oppool, opbufs, optag)
                        for t in range(4) for n in range(4)]
            return pieces, op_factory

        # ================= schedule =================
        with ExitStack() as s1:
            pqkv = s1.enter_context(
                tc.tile_pool(name="pqkv", bufs=2, space="PSUM"))
            pmisc = s1.enter_context(
                tc.tile_pool(name="pmisc", bufs=2, space="PSUM"))
            psim = s1.enter_context(
                tc.tile_pool(name="psim", bufs=1, space="PSUM"))
            ppv = s1.enter_context(
                tc.tile_pool(name="ppv", bufs=2, space="PSUM"))

            # x0..x2 are loaded by the prologue; prefetch distance 2 with
            # bufs=3 (prefetching sub+3 would race the current sub's buffer)
            for p in proj_pieces(0, prefetch=None):
                p()
            for p in proj_pieces(1, prefetch=3):
                p()
            for p in proj_pieces(2, prefetch=4):
                p()
            for p in proj_pieces(3, prefetch=5):
                p()
            # brackets: attn(g) + proj slice g+2 (subs 2g+4, 2g+5), g=0..5.
            # Each group's out-projection pieces are pure PE work and are
            # deferred into the NEXT bracket as chain filler.
            pending_mkops = None
            for g in range(NG - 2):
                ap, mkops = attn_pieces(g, psim, 1, ppv, pmisc, "misc", 2,
                                        chunked_out=False, split_exp=True)
                pp = proj_pieces(2 * g + 4, prefetch=2 * g + 6)
                pp += proj_pieces(2 * g + 5, prefetch=2 * g + 7)
                if pending_mkops is not None:
                    pp = pp + pending_mkops(pmisc, 2, "misc")
                for p in _interleave(ap, pp):
                    p()
                pending_mkops = mkops

        with ExitStack() as s2:
            psim2 = s2.enter_context(
                tc.tile_pool(name="psim2", bufs=2, space="PSUM"))
            ppv2 = s2.enter_context(
                tc.tile_pool(name="ppv2", bufs=2, space="PSUM"))
            popz = s2.enter_context(
                tc.tile_pool(name="popz", bufs=2, space="PSUM"))
            a6, mkops6 = attn_pieces(NG - 2, psim2, 2, ppv2, popz, "opz", 2,
                                     chunked_out=True)
            a7, mkops7 = attn_pieces(NG - 1, psim2, 2, ppv2, popz, "opz", 2,
                                     chunked_out=True)
            # attn(6) with ops(5) as filler, then attn(7) with ops(6).
            # (Fully interleaving the two groups deadlocks: 4 live PV
            # accumulators vs 2 banks.)
            ops5 = pending_mkops(popz, 2, "opz") if pending_mkops else []
            for p in _interleave(a6, ops5):
                p()
            for p in _interleave(a7, mkops6(popz, 2, "opz")):
                p()
        with ExitStack() as s3:
            # group 7's out-projection alone at the very end: give it 4
            # PSUM banks so the matmul/drain rotation never stalls
            pop3 = s3.enter_context(
                tc.tile_pool(name="pop3", bufs=4, space="PSUM"))
            for p in mkops7(pop3, 4, "op3"):
                p()

    nc.dbg_tiles = {"qT": qT, "kT": kT, "v_sb": v_sb}
    nc.compile()
    return nc


def _host_tables(S: int):
    NSUB = S // 256
    inv = 1.0 / (ROPE_BASE ** (np.arange(0, DH, 2, dtype=np.float64) / DH))
    t = np.arange(S, dtype=np.float64)
    fr = np.outer(t, inv)  # [S, 64]
    cos = np.repeat(np.cos(fr), 2, axis=1).T  # [128, S]
    sin = np.repeat(np.sin(fr), 2, axis=1).T
    cs = np.zeros((128, NSUB * 512), np.float32)
    for sub in range(NSUB):
        cs[:, sub * 512:sub * 512 + 256] = cos[:, sub * 256:(sub + 1) * 256]
        cs[:, sub * 512 + 256:sub * 512 + 512] = sin[:, sub * 256:(sub + 1) * 256]

    PT = np.zeros((DH, DH), np.float32)
    for m in range(DH // 2):
        PT[2 * m + 1, 2 * m] = -1.0
        PT[2 * m, 2 * m + 1] = 1.0
    consts = np.zeros((128, 2304), np.float32)
    consts[:, 0:128] = PT
    consts[:, 128:256] = 1.0
    # causal masks for the two diagonal pair tiles: regions [0:512] and
    # [512:1024] hold key blocks (4g+2d) and (4g+2d+1); keep iff
    # query_col >= key_part + 128*(2d+j)
    p = np.arange(128)[:, None]
    c = np.arange(512)[None, :]
    for d in range(2):
        m0 = (c >= p + 256 * d).astype(np.float32)
        m1 = (c >= p + 256 * d + 128).astype(np.float32)
        consts[:, 256 + d * 1024:256 + d * 1024 + 512] = m0
        consts[:, 256 + d * 1024 + 512:256 + (d + 1) * 1024] = m1
    return cs.astype(BF), consts.astype(BF)


def _host_inputs(x, wq, wk, wv, wo, S):
    """Per-core input maps."""
    ND = D // 128
    cs, consts = _host_tables(S)
    xT = np.ascontiguousarray(x.T.astype(np.float32))

    in_maps = []
    for c in range(NCORES):
        hsl = slice(c * HPC * DH, (c + 1) * HPC * DH)
        wqT = wq[hsl].T.astype(BF)  # [D, 256]
        wkT = wk[hsl].T.astype(BF)
        wvT = wv[hsl].T.astype(np.float32)
        wqk = np.zeros((128, ND * 512), BF)
        wvh = np.zeros((128, ND * 256), np.float32)
        for d in range(ND):
            wqk[:, d * 512:d * 512 + 256] = wqT[d * 128:(d + 1) * 128]
            wqk[:, d * 512 + 256:d * 512 + 512] = wkT[d * 128:(d + 1) * 128]
            wvh[:, d * 256:(d + 1) * 256] = wvT[d * 128:(d + 1) * 128]
        woT = wo[:, hsl].T.astype(np.float32)  # [256, D]
        wo_sb = np.concatenate([woT[0:128], woT[128:256]], axis=1)  # [128, 2D]
        in_maps.append({
            "xT": xT,
            "wqk": np.ascontiguousarray(wqk),
            "wv": np.ascontiguousarray(wvh),
            "wo": np.ascontiguousarray(wo_sb),
            "cs": cs,
            "consts": consts,
        })
    return in_maps


def kernel(x, mask, wq, wk, wv, wo):
    x = np.asarray(x, dtype=np.float32)
    wq = np.asarray(wq, dtype=np.float32)
    wk = np.asarray(wk, dtype=np.float32)
    wv = np.asarray(wv, dtype=np.float32)
    wo = np.asarray(wo, dtype=np.float32)
    S = x.shape[0]

    if S not in _BUILD_CACHE:
        _BUILD_CACHE[S] = _build(S)
    nc = _BUILD_CACHE[S]

    in_maps = _host_inputs(x, wq, wk, wv, wo, S)
    res = run_bass_kernel_spmd(
        nc, in_maps, core_ids=list(range(NCORES)), trace=TRACE
    )
    global LAST_RESULT
    LAST_RESULT = res
    out = np.zeros((S, D), np.float32)
    for r in res.results:
        out += r["outp"].astype(np.float32)
    return out


# revision 6
# speedup vs baseline: 1.0510x; 1.0135x over previous
"""Trainium2 Bass kernel for nn_Attention_48000554500172 (v2).

16-head causal attention with RoPE (S=4096, D=2048, H=16, DH=128), sharded
over heads across 8 NeuronCores (2 heads/core, tensor parallel). Each core
computes its 2 heads and a partial [S, D] output projection in bf16; the
host upcasts and sums the 8 partials (the all-reduce).

Key design points (vs the 452us v1):
- Mixed-dtype matmuls tuned to the cost model: MOVING operands are f32r
  (>=256 free) wherever possible because a 2-byte moving operand makes the
  compiler emit an InstLdweights per matmul (~38ns of PE sequencer each).
  Stationary operands are bf16 (dtype irrelevant for cost/ldweights).
  f32r inputs (x, wv, wo) are DMA'd straight from HBM - no rounding copies.
- V is projected directly into [s, dh] layout (lhsT = x chunk), killing
  the per-block PE transposes of v1.
- Scores are computed transposed (keys on partitions) in PAIRED 2-bank
  PSUM tiles [128, 1024] (2 key blocks x 512 queries); one wide exp per
  non-diag pair. Causality for the 2 diagonal pairs is one affine_select
  each, whose 2-level iota pattern also zeroes the inter-region junk.
- Softmax denominators: probs pairs are summed into an f32r zacc on DVE
  (serial chain hidden under the exp pipeline), then ONE ones-matrix
  matmul per (g,hh) broadcasts Z across 128 partitions in PSUM: ~8k PE
  rows total vs ~88k for v1's per-block ones-matmuls + bc broadcast.
- Projection and attention are emission-interleaved (attn group g with
  projection slice g+2) so attention's sim->exp->pv latency chains are
  filled with projection matmuls; attention groups 6,7 run in a second
  PSUM scope with a deeper sim pipeline once projection banks free up.
"""
import math
import numpy as np
import ml_dtypes
from contextlib import ExitStack

import concourse.bass as bass
import concourse.tile as tile
from concourse import bacc, mybir
from concourse.bass_utils import run_bass_kernel_spmd

D, H, DH = 2048, 16, 128
NCORES = 8
HPC = H // NCORES  # 2 heads per core
ROPE_BASE = 10000.0
SCALE = 1.0 / math.sqrt(DH)
F32 = mybir.dt.float32
F32R = mybir.dt.float32r
BF16 = mybir.dt.bfloat16
Exp = mybir.ActivationFunctionType.Exp
BF = ml_dtypes.bfloat16

_BUILD_CACHE: dict = {}
TRACE = False
LAST_RESULT = None


def _interleave(a, b):
    """Merge two thunk lists proportionally."""
    out = []
    ia = ib = 0
    while ia < len(a) or ib < len(b):
        fa = ia / len(a) if a else 1.0
        fb = ib / len(b) if b else 1.0
        if ib >= len(b) or (ia < len(a) and fa <= fb):
            out.append(a[ia]); ia += 1
        else:
            out.append(b[ib]); ib += 1
    return out


def _build(S: int):
    assert S % 512 == 0
    ND = D // 128      # 16 contraction chunks
    NSUB = S // 256    # projection subslices
    NG = S // 512      # attention query groups
    NB = S // 128      # key blocks

    nc = bacc.Bacc("TRN2", target_bir_lowering=False, debug=False)

    xT_d = nc.dram_tensor("xT", [D, S], F32R, kind="ExternalInput")
    wqk_d = nc.dram_tensor("wqk", [128, ND * 512], BF16, kind="ExternalInput")
    wv_d = nc.dram_tensor("wv", [128, ND * 256], F32R, kind="ExternalInput")
    wo_d = nc.dram_tensor("wo", [128, 2 * D], F32R, kind="ExternalInput")
    cs_d = nc.dram_tensor("cs", [128, NSUB * 512], BF16, kind="ExternalInput")
    consts_d = nc.dram_tensor("consts", [128, 2304], BF16, kind="ExternalInput")
    out_d = nc.dram_tensor("outp", [S, D], BF16, kind="ExternalOutput")

    with tile.TileContext(nc) as tc, ExitStack() as ctx:
        persist = ctx.enter_context(tc.tile_pool(name="persist", bufs=1))
        work = ctx.enter_context(tc.tile_pool(name="work", bufs=2))

        # ---- persistent SBUF ----
        qT = persist.tile([128, 2 * S], BF16, tag="qT", name="qT")
        kT = persist.tile([128, 2 * S], BF16, tag="kT", name="kT")
        v_sb = persist.tile([128, NB * 256], BF16, tag="v", name="v_sb")
        wqk_sb = persist.tile([128, ND * 512], BF16, tag="wqk", name="wqk_sb")
        wv_sb = persist.tile([128, ND * 256], F32R, tag="wv", name="wv_sb")
        wo_sb = persist.tile([128, 2 * D], F32R, tag="wo", name="wo_sb")
        cs_sb = persist.tile([128, NSUB * 512], BF16, tag="cs", name="cs_sb")
        consts_sb = persist.tile([128, 2304], BF16, tag="cst", name="consts_sb")
        ones_r = persist.tile([128, 128], F32R, tag="ones", name="ones_r")

        xsub_tiles = {}

        def xsub_tile(sub):
            t = work.tile([128, ND * 256], F32R, tag="xsub", bufs=3,
                          name=f"xs{sub}")
            xsub_tiles[sub] = t
            return t

        def dma_xsub(sub, chunks=1, queue=None):
            eng = queue if queue is not None else nc.sync
            t = xsub_tile(sub)
            src = xT_d.ap()[:, sub * 256:(sub + 1) * 256]
            src3 = src.rearrange("(d p) c -> p d c", p=128)
            dst3 = t[:].rearrange("p (d c) -> p d c", d=ND)
            if chunks == 1:
                eng.dma_start(dst3, src3)
            else:
                dper = ND // chunks
                for c in range(chunks):
                    eng.dma_start(
                        dst3[:, c * dper:(c + 1) * dper, :],
                        src3[:, c * dper:(c + 1) * dper, :],
                    )

        def dma_cs(sub):
            nc.sync.dma_start(
                cs_sb[:, sub * 512:(sub + 1) * 512],
                cs_d.ap()[:, sub * 512:(sub + 1) * 512],
            )

        # ---- prologue DMAs (one serial DMA resource; ordered so the
        # first projection matmuls are fed earliest) ----
        WQC = ND * 512 // 4
        WVC = ND * 256 // 4
        nc.sync.dma_start(wqk_sb[:, 0:WQC], wqk_d.ap()[:, 0:WQC])
        dma_xsub(0, chunks=4)
        nc.sync.dma_start(consts_sb[:], consts_d.ap())
        dma_cs(0)
        for c in range(1, 4):
            nc.sync.dma_start(wqk_sb[:, c * WQC:(c + 1) * WQC],
                              wqk_d.ap()[:, c * WQC:(c + 1) * WQC])
        dma_cs(1)
        for c in range(4):
            nc.sync.dma_start(wv_sb[:, c * WVC:(c + 1) * WVC],
                              wv_d.ap()[:, c * WVC:(c + 1) * WVC])
        dma_xsub(1, chunks=2)
        for sub in range(2, 4):
            dma_cs(sub)
        dma_xsub(2)
        for c in range(4):
            q = 2 * D // 4
            nc.sync.dma_start(wo_sb[:, c * q:(c + 1) * q],
                              wo_d.ap()[:, c * q:(c + 1) * q])
        for sub in range(4, NSUB):
            dma_cs(sub)
        with nc.allow_low_precision(reason="f32r ones for Z broadcast matmul"):
            nc.vector.tensor_copy(ones_r[:], consts_sb[:, 128:256])

        qT2 = qT[:].rearrange("p (h s) -> p h s", h=2)
        kT2 = kT[:].rearrange("p (h s) -> p h s", h=2)

        # ================= emission thunk generators =================

        def proj_pieces(sub, prefetch):
            """Projection of subslice `sub` (256 tokens): q,k,v + rope."""
            pieces = []
            xs = xsub_tiles[sub]

            def qk_half(kind, dlo, dhi, acc):
                # acc pair regions: [h0 256 | h1 256]; one bank.
                for d in range(dlo, dhi):
                    for h in range(2):
                        nc.tensor.matmul(
                            acc[:, h * 256:(h + 1) * 256],
                            wqk_sb[:, d * 512 + (2 * kind + h) * 128:
                                   d * 512 + (2 * kind + h) * 128 + 128],
                            xs[:, d * 256:(d + 1) * 256],
                            start=(d == 0 and h == 0),
                            stop=(d == ND - 1 and h == 1),
                            skip_group_check=True,
                        )

            def v_half(dlo, dhi, acc):
                # acc regions: [blk0 (h0|h1) | blk1 (h0|h1)]; lhsT = x chunk.
                for d in range(dlo, dhi):
                    for b in range(2):
                        nc.tensor.matmul(
                            acc[:, b * 256:(b + 1) * 256],
                            xs[:, d * 256 + b * 128:d * 256 + b * 128 + 128],
                            wv_sb[:, d * 256:(d + 1) * 256],
                            start=(d == 0 and b == 0),
                            stop=(d == ND - 1 and b == 1),
                            skip_group_check=True,
                        )

            state = {}

            def pf():
                if prefetch is not None and prefetch < NSUB:
                    dma_xsub(prefetch)
                state["qacc"] = pqkv.tile([128, 512], F32, tag="qkv", bufs=2,
                                          name="qacc")
                qk_half(0, 0, 8, state["qacc"])

            def tin_of(which):
                def f():
                    t = work.tile([128, 512], F32R, tag="tin", bufs=2,
                                  name="tin")
                    nc.scalar.copy(t[:], state[which][:])
                    state["tin_" + which] = t
                return f

            def rope_of(which, dstT2):
                def f():
                    tin = state["tin_" + which]
                    p_ps = pmisc.tile([128, 512], F32, tag="misc", bufs=2,
                                      name="pps")
                    nc.tensor.matmul(p_ps[:], consts_sb[:, 0:128], tin[:],
                                     start=True, stop=True)
                    cos = cs_sb[:, sub * 512:sub * 512 + 256]
                    sin = cs_sb[:, sub * 512 + 256:sub * 512 + 512]
                    with nc.allow_low_precision(reason="f32r rope products"):
                        t1 = work.tile([128, 512], F32R, tag="t1", bufs=1,
                                       name="t1")
                        nc.vector.tensor_mul(t1[:, 0:256], tin[:, 0:256], cos)
                        nc.vector.tensor_mul(t1[:, 256:512], tin[:, 256:512],
                                             cos)
                        t2 = work.tile([128, 512], F32R, tag="t2", bufs=2,
                                       name="t2")
                        nc.gpsimd.tensor_mul(t2[:, 0:256], p_ps[:, 0:256], sin)
                        nc.gpsimd.tensor_mul(t2[:, 256:512], p_ps[:, 256:512],
                                             sin)
                    dst = dstT2[:, :, sub * 256:(sub + 1) * 256]
                    nc.vector.tensor_add(
                        dst,
                        t1[:].rearrange("p (h s) -> p h s", h=2),
                        t2[:].rearrange("p (h s) -> p h s", h=2),
                    )
                return f

            def k1():
                state["kacc"] = pqkv.tile([128, 512], F32, tag="qkv", bufs=2,
                                          name="kacc")
                qk_half(1, 0, 8, state["kacc"])

            def v1():
                state["vacc"] = pqkv.tile([128, 512], F32, tag="qkv", bufs=2,
                                          name="vacc")
                v_half(0, 8, state["vacc"])

            pieces.append(pf)
            pieces.append(lambda: qk_half(0, 8, ND, state["qacc"]))
            pieces.append(tin_of("qacc"))
            pieces.append(k1)
            pieces.append(lambda: qk_half(1, 8, ND, state["kacc"]))
            pieces.append(tin_of("kacc"))
            pieces.append(v1)
            pieces.append(lambda: v_half(8, ND, state["vacc"]))
            pieces.append(rope_of("qacc", qT2))
            pieces.append(rope_of("kacc", kT2))
            pieces.append(lambda: nc.scalar.copy(
                v_sb[:, sub * 512:(sub + 1) * 512], state["vacc"][:]))
            return pieces

        def attn_pieces(g, simpool, simbufs, pvpool, zpool, ztag,
                        zbufs, chunked_out, split_exp=False,
                        drain_engines=("gpsimd", "dve")):
            """Attention group g (512 queries), both heads + out projection.

            The two heads' pair thunks are interleaved so each head's
            exp/mask/Z latency is hidden under the other head's matmuls.
            chunked_out: ship each 512-col output chunk as its own DMA (for
            the final groups, to hide the store in the kernel drain shadow).
            """
            npair = 2 * (g + 1)
            gq = g * 512
            ots = {}
            states = {0: {}, 1: {}}

            def mk_pair(hh, pi):
                st = states[hh]

                def f():
                    if pi == 0:
                        st["pv"] = pvpool.tile([128, 512], F32, tag="pv",
                                               bufs=2, name="pv")
                    diag = pi >= npair - 2
                    pair = simpool.tile([128, 1024], F32, tag="sim",
                                        bufs=simbufs, name="sim")
                    probs = work.tile([128, 1024], BF16, tag="probs",
                                      bufs=5, name="probs")
                    jA, jB = 2 * pi, 2 * pi + 1
                    if not diag:
                        nc.tensor.matmul(
                            pair[:, 0:512],
                            kT[:, hh * S + jA * 128:hh * S + jA * 128 + 128],
                            qT[:, hh * S + gq:hh * S + gq + 512],
                            start=True, stop=True, skip_group_check=True)
                        nc.tensor.matmul(
                            pair[:, 512:1024],
                            kT[:, hh * S + jB * 128:hh * S + jB * 128 + 128],
                            qT[:, hh * S + gq:hh * S + gq + 512],
                            start=True, stop=True, skip_group_check=True)
                        if split_exp:
                            # halves as soon as each sim lands: shorter
                            # WAR latency for the 1-deep scope-1 pipeline
                            nc.scalar.activation(probs[:, 0:512],
                                                 pair[:, 0:512], Exp,
                                                 scale=SCALE)
                            nc.scalar.activation(probs[:, 512:1024],
                                                 pair[:, 512:1024], Exp,
                                                 scale=SCALE)
                        else:
                            nc.scalar.activation(probs[:], pair[:], Exp,
                                                 scale=SCALE)
                        pvA = (0, 0)   # (out offset, probs offset)
                        pvB = (0, 512)
                    else:
                        d = pi - (npair - 2)  # 0 or 1
                        oA, oB = (0, 128) if d == 0 else (256, 384)
                        # zero the never-exp'd junk early (off critical path)
                        if oA > 0:
                            nc.gpsimd.memset(probs[:, 0:oA], 0.0)
                        nc.gpsimd.memset(probs[:, 512:512 + oB], 0.0)
                        nc.tensor.matmul(
                            pair[:, oA:512],
                            kT[:, hh * S + jA * 128:hh * S + jA * 128 + 128],
                            qT[:, hh * S + gq + oA:hh * S + gq + 512],
                            start=True, stop=True, skip_group_check=True)
                        nc.tensor.matmul(
                            pair[:, 512 + oB:1024],
                            kT[:, hh * S + jB * 128:hh * S + jB * 128 + 128],
                            qT[:, hh * S + gq + oB:hh * S + gq + 512],
                            start=True, stop=True, skip_group_check=True)
                        nc.scalar.activation(probs[:, oA:512],
                                             pair[:, oA:512], Exp,
                                             scale=SCALE)
                        nc.scalar.activation(probs[:, 512 + oB:1024],
                                             pair[:, 512 + oB:1024],
                                             Exp, scale=SCALE)
                        # causal mask + junk zeroing via precomputed mask
                        # tiles (DVE 2x); per-half so each PV matmul waits
                        # only its own half's mask
                        nc.vector.tensor_mul(
                            probs[:, 0:512], probs[:, 0:512],
                            consts_sb[:, 256 + d * 1024:256 + d * 1024 + 512])
                        nc.vector.tensor_mul(
                            probs[:, 512:1024], probs[:, 512:1024],
                            consts_sb[:, 768 + d * 1024:768 + d * 1024 + 512])
                        pvA = (oA, oA)
                        pvB = (oB, 512 + oB)
                    # Z: bf16 halves-add (DVE 2x), then f32r accumulate.
                    # The last pair skips the chain; its halves-sum feeds a
                    # second accumulating Z-matmul directly so the in-order
                    # PE never waits for the chain tail.
                    hs = work.tile([128, 512], BF16, tag="hs", bufs=2,
                                   name="hs")
                    nc.vector.tensor_add(hs[:], probs[:, 0:512],
                                         probs[:, 512:1024])
                    with nc.allow_low_precision(reason="f32r Z accum"):
                        if pi == 0:
                            st["zacc"] = work.tile([128, 512], F32R,
                                                   tag="zacc", bufs=2,
                                                   name="zacc")
                            nc.vector.tensor_copy(st["zacc"][:], hs[:])
                        elif pi < npair - 1:
                            nc.vector.tensor_add(st["zacc"][:],
                                                 st["zacc"][:], hs[:])
                        else:
                            st["hs_last"] = hs
                    nc.tensor.matmul(
                        st["pv"][:, pvA[0]:512],
                        v_sb[:, jA * 256 + hh * 128:jA * 256 + hh * 128 + 128],
                        probs[:, pvA[1]:pvA[1] + 512 - pvA[0]],
                        start=(pi == 0), stop=False,
                        skip_group_check=True)
                    nc.tensor.matmul(
                        st["pv"][:, pvB[0]:512],
                        v_sb[:, jB * 256 + hh * 128:jB * 256 + hh * 128 + 128],
                        probs[:, pvB[1]:pvB[1] + 512 - pvB[0]],
                        start=False, stop=(pi == npair - 1),
                        skip_group_check=True)
                return f

            def mk_ztail(hh):
                st = states[hh]

                def f():
                    zbc = zpool.tile([128, 512], F32, tag=ztag, bufs=zbufs,
                                     name="zbc")
                    nc.tensor.matmul(zbc[:], ones_r[:], st["zacc"][:],
                                     start=True, stop=False)
                    nc.tensor.matmul(zbc[:], consts_sb[:, 128:256],
                                     st["hs_last"][:],
                                     start=False, stop=True)
                    recip = work.tile([128, 512], F32, tag="recip", bufs=2,
                                      name="recip")
                    nc.vector.reciprocal(recip[:], zbc[:])
                    ot = work.tile([128, 512], BF16, tag="ot", bufs=4,
                                   name="ot")
                    nc.vector.tensor_mul(ot[:], st["pv"][:], recip[:])
                    ots[hh] = ot
                return f

            pieces = []
            for pi in range(npair):
                pieces.append(mk_pair(0, pi))
                pieces.append(mk_pair(1, pi))
            pieces.append(mk_ztail(0))
            pieces.append(mk_ztail(1))

            # out projection: 4 token-blocks x 4 d-chunks
            osb_state = {}

            def mk_op(t, n, oppool, opbufs, optag):
                def f():
                    if n == 0:
                        osb_state[t] = work.tile([128, D], BF16, tag="osb",
                                                 bufs=2, name="osb")
                    osb = osb_state[t]
                    op = oppool.tile([128, 512], F32, tag=optag, bufs=opbufs,
                                     name="op")
                    for hh in range(2):
                        nc.tensor.matmul(
                            op[:],
                            ots[hh][:, t * 128:(t + 1) * 128],
                            wo_sb[:, hh * D + n * 512:hh * D + (n + 1) * 512],
                            start=(hh == 0), stop=(hh == 1))
                    if (t * 4 + n) % 2 == 0:
                        nc.scalar.copy(osb[:, n * 512:(n + 1) * 512], op[:])
                    else:
                        nc.vector.tensor_copy(osb[:, n * 512:(n + 1) * 512],
                                              op[:])
                    if chunked_out:
                        nc.sync.dma_start(
                            out_d.ap()[g * 512 + t * 128:
                                       g * 512 + (t + 1) * 128,
                                       n * 512:(n + 1) * 512],
                            osb[:, n * 512:(n + 1) * 512])
                    elif n == 3:
                        nc.sync.dma_start(
                            out_d.ap()[g * 512 + t * 128:
                                       g * 512 + (t + 1) * 128, :],
                            osb[:])
                return f

            def op_factory(oppool, opbufs, optag):
                return [mk_op(t, n, oppool, opbufs, optag)
                        for t in range(4) for n in range(4)]
            return pieces, op_factory

        # ================= schedule =================
        with ExitStack() as s1:
            pqkv = s1.enter_context(
                tc.tile_pool(name="pqkv", bufs=2, space="PSUM"))
            pmisc = s1.enter_context(
                tc.tile_pool(name="pmisc", bufs=2, space="PSUM"))
            psim = s1.enter_context(
                tc.tile_pool(name="psim", bufs=1, space="PSUM"))
            ppv = s1.enter_context(
                tc.tile_pool(name="ppv", bufs=2, space="PSUM"))

            # x0..x2 are loaded by the prologue; prefetch distance 2 with
            # bufs=3 (prefetching sub+3 would race the current sub's buffer)
            for p in proj_pieces(0, prefetch=None):
                p()
            for p in proj_pieces(1, prefetch=3):
                p()
            for p in proj_pieces(2, prefetch=4):
                p()
            for p in proj_pieces(3, prefetch=5):
                p()
            # brackets: attn(g) + proj slice g+2 (subs 2g+4, 2g+5), g=0..5.
            # Each group's out-projection pieces are pure PE work and are
            # deferred into the NEXT bracket as chain filler.
            pending_mkops = None
            for g in range(NG - 2):
                ap, mkops = attn_pieces(g, psim, 1, ppv, pmisc, "misc", 2,
                                        chunked_out=False, split_exp=True)
                pp = proj_pieces(2 * g + 4, prefetch=2 * g + 6)
                pp += proj_pieces(2 * g + 5, prefetch=2 * g + 7)
                if pending_mkops is not None:
                    pp = pp + pending_mkops(pmisc, 2, "misc")
                for p in _interleave(ap, pp):
                    p()
                pending_mkops = mkops

        with ExitStack() as s2:
            psim2 = s2.enter_context(
                tc.tile_pool(name="psim2", bufs=2, space="PSUM"))
            ppv2 = s2.enter_context(
                tc.tile_pool(name="ppv2", bufs=2, space="PSUM"))
            popz = s2.enter_context(
                tc.tile_pool(name="popz", bufs=2, space="PSUM"))
            a6, mkops6 = attn_pieces(NG - 2, psim2, 2, ppv2, popz, "opz", 2,
                                     chunked_out=True)
            a7, mkops7 = attn_pieces(NG - 1, psim2, 2, ppv2, popz, "opz", 2,
                                     chunked_out=True)
            # attn(6) with ops(5) as filler, then attn(7) with ops(6).
            # (Fully interleaving the two groups deadlocks: 4 live PV
            # accumulators vs 2 banks.)
            ops5 = pending_mkops(popz, 2, "opz") if pending_mkops else []
            for p in _interleave(a6, ops5):
                p()
            for p in _interleave(a7, mkops6(popz, 2, "opz")):
                p()
        with ExitStack() as s3:
            # group 7's out-projection alone at the very end: give it 4
            # PSUM banks so the matmul/drain rotation never stalls
            pop3 = s3.enter_context(
                tc.tile_pool(name="pop3", bufs=4, space="PSUM"))
            for p in mkops7(pop3, 4, "op3"):
                p()

    nc.dbg_tiles = {"qT": qT, "kT": kT, "v_sb": v_sb}
    nc.compile()
    return nc


def _host_tables(S: int):
    NSUB = S // 256
    inv = 1.0 / (ROPE_BASE ** (np.arange(0, DH, 2, dtype=np.float64) / DH))
    t = np.arange(S, dtype=np.float64)
    fr = np.outer(t, inv)  # [S, 64]
    cos = np.repeat(np.cos(fr), 2, axis=1).T  # [128, S]
    sin = np.repeat(np.sin(fr), 2, axis=1).T
    cs = np.zeros((128, NSUB * 512), np.float32)
    for sub in range(NSUB):
        cs[:, sub * 512:sub * 512 + 256] = cos[:, sub * 256:(sub + 1) * 256]
        cs[:, sub * 512 + 256:sub * 512 + 512] = sin[:, sub * 256:(sub + 1) * 256]

    PT = np.zeros((DH, DH), np.float32)
    for m in range(DH // 2):
        PT[2 * m + 1, 2 * m] = -1.0
        PT[2 * m, 2 * m + 1] = 1.0
    consts = np.zeros((128, 2304), np.float32)
    consts[:, 0:128] = PT
    consts[:, 128:256] = 1.0
    # causal masks for the two diagonal pair tiles: regions [0:512] and
    # [512:1024] hold key blocks (4g+2d) and (4g+2d+1); keep iff
    # query_col >= key_part + 128*(2d+j)
    p = np.arange(128)[:, None]
    c = np.arange(512)[None, :]
    for d in range(2):
        m0 = (c >= p + 256 * d).astype(np.float32)
        m1 = (c >= p + 256 * d + 128).astype(np.float32)
        consts[:, 256 + d * 1024:256 + d * 1024 + 512] = m0
        consts[:, 256 + d * 1024 + 512:256 + (d + 1) * 1024] = m1
    return cs.astype(BF), consts.astype(BF)


def _host_inputs(x, wq, wk, wv, wo, S):
    """Per-core input maps."""
    ND = D // 128
    cs, consts = _host_tables(S)
    xT = np.ascontiguousarray(x.T.astype(np.float32))

    in_maps = []
    for c in range(NCORES):
        hsl = slice(c * HPC * DH, (c + 1) * HPC * DH)
        wqT = wq[hsl].T.astype(BF)  # [D, 256]
        wkT = wk[hsl].T.astype(BF)
        wvT = wv[hsl].T.astype(np.float32)
        wqk = np.zeros((128, ND * 512), BF)
        wvh = np.zeros((128, ND * 256), np.float32)
        for d in range(ND):
            wqk[:, d * 512:d * 512 + 256] = wqT[d * 128:(d + 1) * 128]
            wqk[:, d * 512 + 256:d * 512 + 512] = wkT[d * 128:(d + 1) * 128]
            wvh[:, d * 256:(d + 1) * 256] = wvT[d * 128:(d + 1) * 128]
        woT = wo[:, hsl].T.astype(np.float32)  # [256, D]
        wo_sb = np.concatenate([woT[0:128], woT[128:256]], axis=1)  # [128, 2D]
        in_maps.append({
            "xT": xT,
            "wqk": np.ascontiguousarray(wqk),
            "wv": np.ascontiguousarray(wvh),
            "wo": np.ascontiguousarray(wo_sb),
            "cs": cs,
            "consts": consts,
        })
    return in_maps


def kernel(x, mask, wq, wk, wv, wo):
    x = np.asarray(x, dtype=np.float32)
    wq = np.asarray(wq, dtype=np.float32)
    wk = np.asarray(wk, dtype=np.float32)
    wv = np.asarray(wv, dtype=np.float32)
    wo = np.asarray(wo, dtype=np.float32)
    S = x.shape[0]

    if S not in _BUILD_CACHE:
        _BUILD_CACHE[S] = _build(S)
    nc = _BUILD_CACHE[S]

    in_maps = _host_inputs(x, wq, wk, wv, wo, S)
    res = run_bass_kernel_spmd(
        nc, in_maps, core_ids=list(range(NCORES)), trace=TRACE
    )
    global LAST_RESULT
    LAST_RESULT = res
    out = np.zeros((S, D), np.float32)
    for r in res.results:
        out += r["outp"].astype(np.float32)
    return out
